# revision 30
# baseline (speedup 1.0000x reference)
"""Trainium2 Bass kernel for nn_BipartiteGraph1d (gnn_message_passing).

Reference computation (N=16384 rows, D=1024 features, L=num_layers=8):
    history[0] = x
    for i in 1..L-1:
        y = mean_j( history[j] @ m(i,j) )   j in 0..i-1, k = i-j-1
            m(i,j) = att_source[k]                    (i even, j even)
                     target_to_source * att_source[k] (i even, j odd)
                     source_to_target * att_target[k] (i odd,  j even)
                     att_target[k]                    (i odd,  j odd)
        history.append(layernorm(relu(y)))
    out = stack(history[-2:])                         (2, N, D)

Strategy (8 NeuronCores, data-parallel over rows), bf16 matmul path:
  * each core gets 2048 rows, processed in 512-row blocks whose full layer
    history lives in SBUF as PE-transposed BF16 tiles hT[j] = h_j.T.
  * ALL weight matrices (att stacks and their products with
    source_to_target / target_to_source) are used in BF16: half the HBM
    traffic of fp32, FWL halves LDWEIGHTS. End-to-end rel-err of the bf16
    pipeline vs the fp32 reference is ~1.3e-2 (within the 2e-2 gate).
  * block 0 streams each matrix's FIRST use straight from the fp32
    sources (gpsimd cast-DMA for the raw att matrices; f32 load + DVE
    multiply against an SBUF-cached bf16 multiplier for the derived
    ones), writing the BF16 DRAM scratch as a side effect. Later uses and
    blocks 1+ stream the scratch. The PE is fed from t=0; no precompute
    round trip on the critical path.
  * each layer accumulates in TWO half-width passes (dout 0:512, 512:1024)
    of 4 PSUM banks each, drawn from a 6-buffer rotation; the other 2
    banks are a dedicated pool for PE-transpose outputs. With the
    rotation, a pass never waits on a bank that was not already drained.
  * layer i's LN outputs are transposed into the history by PE matmuls
    that are INTERLEAVED into layer i+1's weight-chunk stream (and the
    next block's x transposes into layer L-1), so the serial
    relu->stats->normalize chain hides under real matmuls instead of
    stalling the in-order PE queue at every layer boundary.
  * relu+layernorm run natively per-row on ACT/DVE in fp32; the LN output
    is written in BF16 (fp32 only for the two output layers) and
    transposed in bf16 (half the PE cost of f32r transposes).
"""

import numpy as np

_CACHE = {}

TAILKC = 3       # k-chunks of the last j run r-outer (boundary overlap)


def _build(L, rows_per_core, D, S, block, num_devices):
    import concourse.tile as tile
    import concourse.mybir as mybir
    from concourse import bacc
    from contextlib import ExitStack

    F32R = mybir.dt.float32r
    F32 = mybir.dt.float32
    BF16 = mybir.dt.bfloat16
    Relu = mybir.ActivationFunctionType.Relu
    Sqrt = mybir.ActivationFunctionType.Sqrt

    assert D == 1024, "layout hardcodes D=1024"
    assert rows_per_core % block == 0 and block % 128 == 0
    assert 2 <= L <= S + 1
    KC = D // 128          # contraction chunks per matrix
    RC = block // 128      # row chunks per block
    NBLK = rows_per_core // block
    HW = 512               # half-width pass (one psum bank)

    nc = bacc.Bacc("TRN2", target_bir_lowering=False, debug=False,
                   num_devices=num_devices)
    x_d = nc.dram_tensor("x", [rows_per_core, D], F32R, kind="ExternalInput").ap()
    s2t_d = nc.dram_tensor("source_to_target", [D, D], F32R, kind="ExternalInput").ap()
    t2s_d = nc.dram_tensor("target_to_source", [D, D], F32R, kind="ExternalInput").ap()
    As_d = nc.dram_tensor("att_source", [S, D, D], F32R, kind="ExternalInput").ap()
    At_d = nc.dram_tensor("att_target", [S, D, D], F32R, kind="ExternalInput").ap()
    id_d = nc.dram_tensor("ident", [128, 128], F32R, kind="ExternalInput").ap()
    out_d = nc.dram_tensor("out", [2, rows_per_core, D], F32R,
                           kind="ExternalOutput").ap()

    # matrix kinds (k = i-j-1, always < S here):
    #   derived b[k] = t2s * As[k]   (i even, j odd)
    #   derived c[k] = s2t * At[k]   (i odd,  j even)
    #   direct  As[k]                (i even, j even)
    #   direct  At[k]                (i odd,  j odd)
    def mkind(i, j):
        if i % 2 == 0:
            return "s" if j % 2 == 0 else "b"
        return "c" if j % 2 == 0 else "t"

    need = {}
    for i in range(1, L):
        for j in range(i):
            need.setdefault((mkind(i, j), i - j - 1), i)   # first-use layer

    with tile.TileContext(nc) as tc, ExitStack() as ctx:
        cst = ctx.enter_context(tc.tile_pool(name="cst", bufs=1))
        hist = ctx.enter_context(tc.tile_pool(name="hist", bufs=1))
        wp = ctx.enter_context(tc.tile_pool(name="wp", bufs=12))
        zp = ctx.enter_context(tc.tile_pool(name="zp", bufs=6))
        hp = ctx.enter_context(tc.tile_pool(name="hp", bufs=3))
        hpb = ctx.enter_context(tc.tile_pool(name="hpb", bufs=6))
        sp = ctx.enter_context(tc.tile_pool(name="sp", bufs=6))
        pp = ctx.enter_context(tc.tile_pool(name="pp", bufs=6))
        xp = ctx.enter_context(tc.tile_pool(name="xp", bufs=4))
        ps = ctx.enter_context(tc.tile_pool(name="ps", bufs=7, space="PSUM"))
        tpp = ctx.enter_context(tc.tile_pool(name="tpp", bufs=1, space="PSUM"))
        dramp = ctx.enter_context(tc.tile_pool(name="dramp", bufs=1, space="DRAM"))

        identb = cst.tile([128, 128], BF16)
        nc.gpsimd.dma_start(identb, id_d)        # cast f32 -> bf16
        eps_t = cst.tile([128, 1], F32)
        nc.vector.memset(eps_t, 1e-5)

        # block-0 x loads first (cast to bf16 on the gpsimd SWDGE ring)
        next_x_tiles = []
        for r in range(RC):
            xt0 = xp.tile([128, D], BF16, tag="x", name=f"x0_{r}")
            nc.gpsimd.dma_start(xt0, x_d[r * 128:(r + 1) * 128, :])
            next_x_tiles.append(xt0)

        # SBUF-cached bf16 copies of the two multiplier matrices. Layers
        # 1-2 use per-chunk f32 mult loads instead (scalar ring), so the
        # cache loads are DEFERRED to ~2 layers before their first real
        # consumer - 8MB of multb traffic ahead of the layer-1/2 streams
        # on the gpsimd ring was gating the block-0 ramp.
        multb = {}
        multb_emit = {}
        for kd in ("c", "b"):
            fus = [fu for (kd2, _), fu in need.items() if kd2 == kd and fu > 2]
            if fus:
                multb[kd] = cst.tile([128, KC, D], BF16, name=f"mult{kd}")
                multb_emit.setdefault(max(1, min(fus) - 2), []).append(kd)

        def load_multb(kd):
            src = s2t_d if kd == "c" else t2s_d
            for kc in range(KC):
                nc.gpsimd.dma_start(multb[kd][:, kc, :],
                                    src[kc * 128:(kc + 1) * 128, :])

        # one BF16 DRAM scratch tile per matrix, stored CHUNK-MAJOR
        # [kc, half, 128, HW] so every steady-state weight read is one
        # fully contiguous 128KB block (strided 1KB row segments measured
        # ~214GB/s effective; contiguous reads restore line rate).
        # Dependency tracking is per-matrix, so a layer only waits for
        # the matrix it reads.
        mat_t = {key: dramp.tile([KC, 2, 128, HW], BF16,
                                 tag=f"{key[0]}m{key[1]}",
                                 name=f"{key[0]}m{key[1]}")
                 for key in need}

        def load_w(i, j, kc, half, b):
            """Weight chunk [128, HW] bf16 for (i, j), chunk kc, dout half.

            Block-0 first uses stream from the fp32 sources on the scalar
            HWDGE ring (f32 load + DVE convert/multiply) and write the
            BF16 scratch as a side effect - so the scratch needs no
            separate conversion pass and the gpsimd ring stays clear for
            x / out / multb traffic."""
            kd, k = mkind(i, j), i - j - 1
            cols = slice(half * HW, (half + 1) * HW)
            w_t = wp.tile([128, HW], BF16, tag="w", name="wt")
            if b == 0 and need[kd, k] == i:
                att = As_d if kd in ("s", "b") else At_d
                src = att[k, kc * 128:(kc + 1) * 128, cols]
                if kd in ("s", "t"):
                    # direct first use: gpsimd cast-DMA; scratch write on
                    # the sync HWDGE ring (cross-engine ordering - a
                    # second SWDGE DMA chained off the cast's destination
                    # showed nondeterministic corruption)
                    nc.gpsimd.dma_start(w_t, src)
                    nc.sync.dma_start(mat_t[kd, k][kc, half], w_t)
                else:
                    # derived first use: f32 load shares the sync ring
                    # (block 0 has few scratch reads), DVE multiply
                    a_t = pp.tile([128, HW], F32R, tag="pre", name="pa")
                    nc.sync.dma_start(a_t, src)
                    if i <= 2:
                        # layers 1-2: per-chunk mult load on the (idle)
                        # scalar ring, not the deferred multb cache
                        mult = s2t_d if kd == "c" else t2s_d
                        m_t = pp.tile([128, HW], F32R, tag="pre", name="pm")
                        nc.scalar.dma_start(m_t,
                                            mult[kc * 128:(kc + 1) * 128, cols])
                        nc.vector.tensor_mul(w_t, a_t, m_t)
                    else:
                        nc.vector.tensor_mul(w_t, a_t, multb[kd][:, kc, cols])
                    nc.gpsimd.dma_start(mat_t[kd, k][kc, half], w_t)
            else:
                nc.sync.dma_start(w_t, mat_t[kd, k][kc, half])
            return w_t

        hT0_next = None
        for b in range(NBLK):
            if hT0_next is not None:
                hT = [hT0_next]
            else:
                hT = [hist.tile([128, KC, block], BF16, tag="hT0", name="hT0")]
            hT += [hist.tile([128, KC, block], BF16, tag=f"hT{j}", name=f"hT{j}")
                   for j in range(1, L - 1)]
            hT0_next = None

            def tp_group(dst_hT, src_tile, r, half):
                # PE-transpose src[:, half*512:(half+1)*512] bf16 into
                # dst[:, dc, r*128:(r+1)*128] for 4 dc, via one psum tile
                tp = tpp.tile([128, 4, 128], BF16, tag="tp", name="tp")
                for q in range(4):
                    dc = half * 4 + q
                    nc.tensor.transpose(
                        tp[:, q, :], src_tile[:, dc * 128:(dc + 1) * 128],
                        identb)
                nc.scalar.copy(
                    dst_hT[:, half * 4:half * 4 + 4, r * 128:(r + 1) * 128],
                    tp)

            def tp_groups(dst_hT, src_tile, r):
                for half in range(KC // 4):
                    tp_group(dst_hT, src_tile, r, half)

            # block-0 (and L==2, where layer L-1 cannot host the
            # interleave): x transposes happen right here
            if b == 0 or L == 2:
                for r in range(RC):
                    tp_groups(hT[0], next_x_tiles[r], r)

            pending_tp = []    # [(dst_hT, src_bf16_tile, r)] from layer i-1
            for i in range(1, L):
                if b == 0:
                    # scratch-fed js first, then first-use direct, then
                    # first-use derived (DVE pipeline), freshest history
                    # last - maximum slack for the first-use streams.
                    def jkey(j):
                        kd, k = mkind(i, j), i - j - 1
                        fu = need[kd, k] == i
                        return (1 if fu else 0,
                                1 if kd in ("b", "c") else 0, j)
                    js = sorted(range(i - 1), key=jkey)
                    js.append(i - 1)
                else:
                    js = list(range(i))
                jlast = js[-1]
                # deeper stagger on the very last layer: its LN/out chain
                # has no following matmuls to hide under
                tailkc = min(2 * TAILKC, KC) \
                    if (b == NBLK - 1 and i == L - 1) else TAILKC
                head = [(j, kc) for j in js for kc in range(KC)
                        if not (j == jlast and kc >= KC - tailkc)]

                # interleave schedule for deferred transposes (they only
                # depend on layer i-1's LN outputs / prefetched x tiles);
                # one 4-dc group per slot so consecutive groups never wait
                # on each other's psum-evacuation copy
                # pending_tp writes hT[i-1], which THIS layer's jlast
                # chunks read - every group must be emitted strictly
                # before jlast's first head chunk (position cap), else the
                # reads are emitted first and consume stale history.
                jlast_start = (len(js) - 1) * KC
                tp_sched = {}
                for idx, (dst, src, r2) in enumerate(pending_tp):
                    for g in range(2):
                        pos = min(2 + 3 * idx + g, jlast_start - 1,
                                  len(head) - 1)
                        tp_sched.setdefault(pos, []).append((dst, src, r2, g))
                # next block's x transposes reuse the hT0 BUFFER (tag
                # rotation, bufs=1) while the old hT0 tile still has
                # readers in this layer's LATER emission - so they must be
                # interleaved into the LAST pass (half==1), after its j=0
                # chunks; every old-hT0 read is emitted before them then.
                tp_sched_p1 = {}
                if i == L - 1 and b + 1 < NBLK and L > 2:
                    j0_end = (js.index(0) + 1) * KC
                    hT0_next = hist.tile([128, KC, block], BF16, tag="hT0",
                                         name="hT0")
                    for idx, r2 in enumerate(range(RC)):
                        for g in range(2):
                            pos = min(j0_end + 1 + 3 * idx + g, len(head) - 1)
                            tp_sched_p1.setdefault(pos, []).append(
                                (hT0_next, next_x_tiles[r2], r2, g))

                z = [None] * RC
                for half in range(2):
                    y = [ps.tile([128, HW], F32, tag="acc", name=f"y{r}")
                         for r in range(RC)]
                    for n, (j, kc) in enumerate(head):
                        w_t = load_w(i, j, kc, half, b)
                        for r in range(RC):
                            nc.tensor.matmul(
                                y[r], lhsT=hT[j][:, kc, r * 128:(r + 1) * 128],
                                rhs=w_t, start=(n == 0), stop=False)
                        sched = tp_sched if half == 0 else tp_sched_p1
                        for dst, src, r2, g in sched.get(n, []):
                            tp_group(dst, src, r2, g)
                    wtail = {kc: load_w(i, jlast, kc, half, b)
                             for kc in range(KC - tailkc, KC)}
                    for r in range(RC):
                        for kc in range(KC - tailkc, KC):
                            nc.tensor.matmul(
                                y[r], lhsT=hT[jlast][:, kc, r * 128:(r + 1) * 128],
                                rhs=wtail[kc], start=False, stop=(kc == KC - 1))
                    inv = 1.0 / i
                    for r in range(RC):
                        if half == 0:
                            z[r] = zp.tile([128, D], F32, tag="z", name="z")
                        nc.scalar.activation(z[r][:, half * HW:(half + 1) * HW],
                                             y[r], Relu, scale=inv)

                pending_tp = []
                for r in range(RC):
                    st = sp.tile([128, 2, 6], F32, tag="st", name="st")
                    for c in range(2):
                        nc.vector.bn_stats(st[:, c, :], z[r][:, c * HW:(c + 1) * HW])
                    mv = sp.tile([128, 2], F32, tag="mv", name="mv")
                    nc.vector.bn_aggr(mv, st)
                    rstd = sp.tile([128, 1], F32, tag="rs", name="rs")
                    nc.scalar.activation(rstd, mv[:, 1:2], Sqrt, bias=eps_t)
                    nc.vector.reciprocal(rstd, rstd)
                    oi = i - (L - 2)
                    if oi >= 0:
                        h32 = hp.tile([128, D], F32R, tag="h32", name="h32")
                        nc.vector.tensor_scalar(
                            out=h32, in0=z[r], scalar1=mv[:, 0:1], scalar2=rstd,
                            op0=mybir.AluOpType.subtract, op1=mybir.AluOpType.mult)
                        row0 = b * block + r * 128
                        # final block: HWDGE (lower latency) for the
                        # drain-critical last stores
                        oeng = nc.scalar if b == NBLK - 1 else nc.gpsimd
                        oeng.dma_start(out_d[oi, row0:row0 + 128, :], h32)
                        if i < L - 1:
                            h_bf = hpb.tile([128, D], BF16, tag="hb", name="hb")
                            nc.vector.tensor_copy(h_bf, h32)
                            pending_tp.append((hT[i], h_bf, r))
                    else:
                        h_bf = hpb.tile([128, D], BF16, tag="hb", name="hb")
                        nc.vector.tensor_scalar(
                            out=h_bf, in0=z[r], scalar1=mv[:, 0:1], scalar2=rstd,
                            op0=mybir.AluOpType.subtract, op1=mybir.AluOpType.mult)
                        pending_tp.append((hT[i], h_bf, r))

                # deferred multb cache loads (block 0 only)
                if b == 0:
                    for kd in multb_emit.get(i, []):
                        load_multb(kd)

                # prefetch the next block's x tiles well ahead of their use
                if i == max(1, L - 2) and b + 1 < NBLK:
                    next_x_tiles = []
                    for r in range(RC):
                        xt = xp.tile([128, D], BF16, tag="x", name="xt")
                        row0 = (b + 1) * block + r * 128
                        nc.gpsimd.dma_start(xt, x_d[row0:row0 + 128, :])
                        next_x_tiles.append(xt)

    nc.compile()
    return nc


def kernel(x, source_to_target, target_to_source, att_source, att_target,
           num_layers):
    from concourse.bass_utils import run_bass_kernel_spmd

    x = np.ascontiguousarray(np.asarray(x, dtype=np.float32))
    s2t = np.ascontiguousarray(np.asarray(source_to_target, dtype=np.float32))
    t2s = np.ascontiguousarray(np.asarray(target_to_source, dtype=np.float32))
    As = np.ascontiguousarray(np.asarray(att_source, dtype=np.float32))
    At = np.ascontiguousarray(np.asarray(att_target, dtype=np.float32))
    L = int(num_layers)

    N, D = x.shape
    S = As.shape[0]
    n_cores = 8
    assert N % n_cores == 0
    rows = N // n_cores
    block = 512 if rows % 512 == 0 else 128

    key = (L, rows, D, S, block, n_cores)
    if key not in _CACHE:
        _CACHE[key] = _build(L, rows, D, S, block, n_cores)
    nc = _CACHE[key]

    ident = np.eye(128, dtype=np.float32)
    in_maps = [
        {
            "x": x[c * rows:(c + 1) * rows],
            "source_to_target": s2t,
            "target_to_source": t2s,
            "att_source": As,
            "att_target": At,
            "ident": ident,
        }
        for c in range(n_cores)
    ]
    res = run_bass_kernel_spmd(nc, in_maps, list(range(n_cores))).results
    out = np.concatenate([res[c]["out"] for c in range(n_cores)], axis=1)
    if L == 2:
        out[0] = x  # history[-2] is the input itself
    return out.astype(np.float32, copy=False)
